# revision 34
# baseline (speedup 1.0000x reference)
"""Multi-head attention kernel for Trainium2, SPMD over 8 NeuronCores.

Problem: qkv (8, 1536, 2048) f32 -> out (8, 512, 2048) f32
  B=8 batches, H=8 heads, C=64 channels/head, T=2048 tokens.
  out[b] = concat_h( softmax((q_h*s)^T (k_h*s)) applied to v_h )
  with s = C**-0.25 (i.e. scores scaled by C**-0.5 overall).

Sharding: batch b -> core b. Each core computes 8 heads independently;
no collectives needed.

Per-head algorithm on one core (all on-chip):
  for each s-chunk (128 keys):
    scoresT[s,t] = sum_c k[c,s] q[c,t]        (PE, f32)
    pT[s,t] = exp(0.125 * scoresT)            (ACT, bf16 out, no max-sub:
                                               scores ~ N(0,1), safe in f32)
    acc[c,t] += vT_ones[s, c] * pT[s,t]       (PE, bf16; row 64 of vT_ones
                                               is ones -> acc[64,t] = l[t])
  out[c,t] = acc[c,t] / l[t]                  (DVE + DMA broadcast of 1/l)
"""

import os
import sys

import numpy as np

for _p in ("/opt/trn_rl_repo", "/root/.axon_site/_ro/trn_rl_repo"):
    if os.path.isdir(_p) and _p not in sys.path:
        sys.path.insert(0, _p)

B, H, C, T = 8, 8, 64, 2048
HC = H * C  # 512
W = 3 * HC  # 1536
NCH = T // 128  # 16 key chunks of 128
THALF = T // 2  # 1024

# Schraudolph exp -> bf16 bits via an int16 convert; softmax
# normalization cancels the approximation's common-mode bias
# (end-to-end rel err ~1.1e-2 < 2e-2).
SCH_A = 0.125 * 128 * 1.4426950408889634
SCH_B = 16256.0 - 5.5 + 0.5

_CACHE = {}


def _build_nc():
    from contextlib import ExitStack

    import concourse.bass as bass
    import concourse.mybir as mybir
    from concourse import bacc
    from concourse.masks import make_identity
    from concourse.tile import TileContext

    f32 = mybir.dt.float32
    bf16 = mybir.dt.bfloat16
    i16 = mybir.dt.int16
    Exp = mybir.ActivationFunctionType.Exp
    mul_op = mybir.AluOpType.mult
    add_op = mybir.AluOpType.add

    nc = bacc.Bacc("TRN2", target_bir_lowering=False, debug=False)
    qkv = nc.declare_dram_parameter("qkv", [W, T], f32, isOutput=False)
    out = nc.declare_dram_parameter("out", [HC, T], f32, isOutput=True)

    with TileContext(nc) as tc, ExitStack() as ctx:
        singles = ctx.enter_context(tc.tile_pool(name="singles", bufs=1))
        qkv_pool = ctx.enter_context(tc.tile_pool(name="qkvp", bufs=2))
        vt_pool = ctx.enter_context(tc.tile_pool(name="vtp", bufs=2))
        pt_pool = ctx.enter_context(tc.tile_pool(name="ptp", bufs=10))
        out_pool = ctx.enter_context(tc.tile_pool(name="outp", bufs=2))
        l_pool = ctx.enter_context(tc.tile_pool(name="lp", bufs=2))
        ps_sc = ctx.enter_context(tc.tile_pool(name="ps_sc", bufs=2, space="PSUM"))
        ps_av = ctx.enter_context(tc.tile_pool(name="ps_av", bufs=1, space="PSUM"))

        # identity for PE transposes, one copy per partition half so the
        # rhs base partition matches lhsT for both heads of a pair
        ident = singles.tile([128, 64], f32)
        make_identity(nc, ident[0:64, :])
        make_identity(nc, ident[64:128, :])


        for pair in range(4):
            q2 = qkv_pool.tile([128, T], f32, tag="q2")
            k2 = qkv_pool.tile([128, T], f32, tag="k2")
            v2 = qkv_pool.tile([128, T], f32, tag="v2")
            q2b = qkv_pool.tile([128, T], bf16, tag="q2b")
            k2b = qkv_pool.tile([128, T], bf16, tag="k2b")
            r0 = pair * 128
            if pair == 0:
                # load + cast only what QK_0/exp_0 need first (32KB of
                # k, half of q), then the rest: first exp fires earlier
                nc.sync.dma_start(out=k2[0:64, 0:128], in_=qkv[HC : HC + 64, 0:128])
                nc.sync.dma_start(out=q2[0:64, 0:THALF], in_=qkv[0:64, 0:THALF])
                nc.scalar.copy(k2b[0:64, 0:128], k2[0:64, 0:128])
                nc.scalar.copy(q2b[0:64, 0:THALF], q2[0:64, 0:THALF])
                nc.sync.dma_start(out=k2[0:64, 128:T], in_=qkv[HC : HC + 64, 128:T])
                nc.sync.dma_start(out=q2[0:64, THALF:T], in_=qkv[0:64, THALF:T])
                nc.scalar.copy(k2b[0:64, 128:T], k2[0:64, 128:T])
                nc.scalar.copy(q2b[0:64, THALF:T], q2[0:64, THALF:T])
                nc.sync.dma_start(out=v2, in_=qkv[2 * HC : 2 * HC + 128, :])
                nc.sync.dma_start(out=k2[64:128, :], in_=qkv[HC + 64 : HC + 128, :])
                nc.sync.dma_start(out=q2[64:128, :], in_=qkv[64:128, :])
                nc.scalar.copy(k2b[64:128, :], k2[64:128, :])
                nc.scalar.copy(q2b[64:128, :], q2[64:128, :])
            else:
                nc.sync.dma_start(out=q2, in_=qkv[r0 : r0 + 128, :])
                nc.sync.dma_start(out=k2, in_=qkv[HC + r0 : HC + r0 + 128, :])
                nc.sync.dma_start(
                    out=v2, in_=qkv[2 * HC + r0 : 2 * HC + r0 + 128, :]
                )
                # bf16 q/k: matmul streams 1 col/cycle vs 2 for f32;
                # casts on ACT keep the DVE exp stream unobstructed
                nc.scalar.copy(q2b, q2)
                nc.scalar.copy(k2b, k2)

            for hh in range(2):
                h = pair * 2 + hh
                o = hh * 64
                q = q2b[o : o + 64, :]
                k = k2b[o : o + 64, :]
                v = v2[o : o + 64, :]

                # v -> [s, c] transposes happen on the PE, but are emitted
                # inside chunks 0-1 below so they sit in PE slack during the
                # first exps instead of blocking QK_0 at the head boundary
                trans = ps_av.tile([128, NCH, 64], f32, tag="av")
                vt = vt_pool.tile([128, NCH, 66], bf16)

                def emit_trans(lo, hi):
                    for j in range(lo, hi):
                        nc.tensor.transpose(
                            trans[:, j, :],
                            v[:, j * 128 : (j + 1) * 128],
                            ident[o : o + 64, :],
                        )
                    if hi == NCH:
                        # vt rows: 0..63 = v, 64 = ones (l via the AV matmul)
                        nc.vector.tensor_copy(vt[:, :, 0:64], trans)
                        nc.vector.memset(vt[:, :, 64:65], 1.0)

                av = ps_av.tile([128, T], f32, tag="av")

                def emit_av(j, pts_j):
                    # one LDW for all 4 AV matmuls of chunk j
                    for half in range(2):
                        t0 = half * THALF
                        for qq in range(2):
                            nc.tensor.matmul(
                                av[0:65, t0 + qq * 512 : t0 + (qq + 1) * 512],
                                vt[:, j, 0:65],
                                pts_j[half][:, qq * 512 : (qq + 1) * 512],
                                start=(j == 0),
                                stop=(j == NCH - 1),
                                skip_group_check=True,
                            )

                # software pipeline: QK(j)+exp(j) stream, AV lags one chunk
                # so the PE can run QK(j+1) between exp(j,lo) and exp(j,hi)
                prev_pts = None
                for j in range(NCH):
                    kj = k[:, j * 128 : (j + 1) * 128]
                    scs = []
                    for half in range(2):
                        t0 = half * THALF
                        sc = ps_sc.tile([128, THALF], f32, tag="sc")
                        scs.append(sc)
                        for qq in range(2):
                            nc.tensor.matmul(
                                sc[:, qq * 512 : (qq + 1) * 512],
                                kj,
                                q[:, t0 + qq * 512 : t0 + (qq + 1) * 512],
                                start=True,
                                stop=True,
                            )
                    pts = []
                    for half in range(2):
                        pt = pt_pool.tile([128, THALF], bf16)
                        pts.append(pt)
                        if j % 2 == 0:
                            nc.scalar.activation(pt, scs[half], Exp, scale=0.125)
                        else:
                            # Schraudolph exp on DVE halves ACT's load so
                            # the AV matmuls stop waiting on softmax
                            nc.vector.tensor_scalar(
                                pt.bitcast(i16), scs[half], SCH_A, SCH_B,
                                mul_op, add_op,
                            )
                    if j == 0:
                        emit_trans(0, NCH // 2)
                    elif j == 1:
                        emit_trans(NCH // 2, NCH)
                    if prev_pts is not None:
                        emit_av(j - 1, prev_pts)
                    prev_pts = pts
                emit_av(NCH - 1, prev_pts)

                # evacuate av to SBUF on ACT (it has slack once exp is
                # split; keeps multi-us work out of the DVE queue so the
                # next head's DVE exps start on time)
                av_sb = out_pool.tile([65, T], f32, tag="avsb")
                nc.scalar.copy(av_sb[:, 0:THALF], av[0:65, 0:THALF])
                nc.scalar.copy(av_sb[:, THALF:T], av[0:65, THALF:T])
                # normalize out = av[0:64] * (1/l): the DVE only stages l
                # to partition 0 and takes the [1,*] reciprocal (short);
                # broadcast + multiply run on the otherwise idle gpsimd
                l_sb = l_pool.tile([1, T], f32, tag="lsb")
                rl1 = l_pool.tile([1, T], f32, tag="rl1")
                rlb = l_pool.tile([64, T], f32, tag="rlb")
                o_sb = out_pool.tile([64, T], f32, tag="osb")
                for half in range(2):
                    t0, t1 = half * THALF, (half + 1) * THALF
                    nc.vector.tensor_copy(l_sb[:, t0:t1], av_sb[64:65, t0:t1])
                    nc.vector.reciprocal_approx_fast(
                        out=rl1[:, t0:t1], in_=l_sb[:, t0:t1]
                    )
                    nc.gpsimd.partition_broadcast(rlb[:, t0:t1], rl1[:, t0:t1])
                    nc.gpsimd.tensor_mul(
                        o_sb[:, t0:t1], av_sb[0:64, t0:t1], rlb[:, t0:t1]
                    )
                    nc.sync.dma_start(
                        out=out[h * 64 : (h + 1) * 64, t0:t1], in_=o_sb[:, t0:t1]
                    )

    nc.finalize()
    return nc


def _get_nc():
    if "nc" not in _CACHE:
        _CACHE["nc"] = _build_nc()
    return _CACHE["nc"]


def _run(qkv_full, trace=False, tmpdir=None):
    """qkv_full: (8, 1536, 2048) f32. Returns (out (8,512,2048) f32, exec_ns)."""
    from concourse.bass_utils import run_bass_kernel_spmd

    nc = _get_nc()
    qkv_full = np.ascontiguousarray(np.asarray(qkv_full, dtype=np.float32))
    in_maps = [{"qkv": qkv_full[i]} for i in range(B)]
    res = run_bass_kernel_spmd(
        nc, in_maps, core_ids=list(range(B)), trace=trace, tmpdir=tmpdir
    )
    outs = np.stack([np.asarray(res.results[i]["out"]) for i in range(B)], axis=0)
    return outs, res.exec_time_ns


def kernel(qkv, n_heads=8):
    out, _ = _run(qkv)
    return out.astype(np.float32)



# revision 36
# speedup vs baseline: 1.0295x; 1.0295x over previous
"""Multi-head attention kernel for Trainium2, SPMD over 8 NeuronCores.

Problem: qkv (8, 1536, 2048) f32 -> out (8, 512, 2048) f32
  B=8 batches, H=8 heads, C=64 channels/head, T=2048 tokens.
  out[b] = concat_h( softmax((q_h*s)^T (k_h*s)) applied to v_h )
  with s = C**-0.25 (i.e. scores scaled by C**-0.5 overall).

Sharding: batch b -> core b. Each core computes 8 heads independently;
no collectives needed.

Per-head algorithm on one core (all on-chip):
  for each s-chunk (128 keys):
    scoresT[s,t] = sum_c k[c,s] q[c,t]        (PE, f32)
    pT[s,t] = exp(0.125 * scoresT)            (ACT, bf16 out, no max-sub:
                                               scores ~ N(0,1), safe in f32)
    acc[c,t] += vT_ones[s, c] * pT[s,t]       (PE, bf16; row 64 of vT_ones
                                               is ones -> acc[64,t] = l[t])
  out[c,t] = acc[c,t] / l[t]                  (DVE + DMA broadcast of 1/l)
"""

import os
import sys

import numpy as np

for _p in ("/opt/trn_rl_repo", "/root/.axon_site/_ro/trn_rl_repo"):
    if os.path.isdir(_p) and _p not in sys.path:
        sys.path.insert(0, _p)

B, H, C, T = 8, 8, 64, 2048
HC = H * C  # 512
W = 3 * HC  # 1536
NCH = T // 128  # 16 key chunks of 128
THALF = T // 2  # 1024

# Schraudolph exp -> bf16 bits via an int16 convert; softmax
# normalization cancels the approximation's common-mode bias
# (end-to-end rel err ~1.1e-2 < 2e-2).
SCH_A = 0.125 * 128 * 1.4426950408889634
SCH_B = 16256.0 - 5.5 + 0.5

_CACHE = {}


def _build_nc():
    from contextlib import ExitStack

    import concourse.bass as bass
    import concourse.mybir as mybir
    from concourse import bacc
    from concourse.masks import make_identity
    from concourse.tile import TileContext

    f32 = mybir.dt.float32
    bf16 = mybir.dt.bfloat16
    i16 = mybir.dt.int16
    Exp = mybir.ActivationFunctionType.Exp
    mul_op = mybir.AluOpType.mult
    add_op = mybir.AluOpType.add

    nc = bacc.Bacc("TRN2", target_bir_lowering=False, debug=False)
    qkv = nc.declare_dram_parameter("qkv", [W, T], f32, isOutput=False)
    out = nc.declare_dram_parameter("out", [HC, T], f32, isOutput=True)

    with TileContext(nc) as tc, ExitStack() as ctx:
        singles = ctx.enter_context(tc.tile_pool(name="singles", bufs=1))
        qkv_pool = ctx.enter_context(tc.tile_pool(name="qkvp", bufs=2))
        vt_pool = ctx.enter_context(tc.tile_pool(name="vtp", bufs=2))
        pt_pool = ctx.enter_context(tc.tile_pool(name="ptp", bufs=10))
        out_pool = ctx.enter_context(tc.tile_pool(name="outp", bufs=2))
        l_pool = ctx.enter_context(tc.tile_pool(name="lp", bufs=2))
        ps_sc = ctx.enter_context(tc.tile_pool(name="ps_sc", bufs=2, space="PSUM"))
        ps_av = ctx.enter_context(tc.tile_pool(name="ps_av", bufs=1, space="PSUM"))

        # identity for PE transposes, one copy per partition half so the
        # rhs base partition matches lhsT for both heads of a pair
        ident = singles.tile([128, 64], f32)
        make_identity(nc, ident[0:64, :])
        make_identity(nc, ident[64:128, :])


        for pair in range(4):
            q2 = qkv_pool.tile([128, T], f32, tag="q2")
            k2 = qkv_pool.tile([128, T], f32, tag="k2")
            v2 = qkv_pool.tile([128, T], f32, tag="v2")
            q2b = qkv_pool.tile([128, T], bf16, tag="q2b")
            k2b = qkv_pool.tile([128, T], bf16, tag="k2b")
            r0 = pair * 128
            if pair == 0:
                # load + cast only what QK_0/exp_0 need first (32KB of
                # k, half of q), then the rest: first exp fires earlier
                # head 0's v half first: the PE transposes (chunks 0-1)
                # are the longest startup chain; then the QK_0 pieces
                nc.sync.dma_start(out=v2[0:64, :], in_=qkv[2 * HC : 2 * HC + 64, :])
                nc.sync.dma_start(out=k2[0:64, 0:128], in_=qkv[HC : HC + 64, 0:128])
                nc.sync.dma_start(out=q2[0:64, 0:THALF], in_=qkv[0:64, 0:THALF])
                nc.scalar.copy(k2b[0:64, 0:128], k2[0:64, 0:128])
                nc.scalar.copy(q2b[0:64, 0:THALF], q2[0:64, 0:THALF])
                nc.sync.dma_start(out=k2[0:64, 128:T], in_=qkv[HC : HC + 64, 128:T])
                nc.sync.dma_start(out=q2[0:64, THALF:T], in_=qkv[0:64, THALF:T])
                nc.scalar.copy(k2b[0:64, 128:T], k2[0:64, 128:T])
                nc.scalar.copy(q2b[0:64, THALF:T], q2[0:64, THALF:T])
                nc.sync.dma_start(out=v2[64:128, :], in_=qkv[2 * HC + 64 : 2 * HC + 128, :])
                nc.sync.dma_start(out=k2[64:128, :], in_=qkv[HC + 64 : HC + 128, :])
                nc.sync.dma_start(out=q2[64:128, :], in_=qkv[64:128, :])
                nc.scalar.copy(k2b[64:128, :], k2[64:128, :])
                nc.scalar.copy(q2b[64:128, :], q2[64:128, :])
            else:
                nc.sync.dma_start(out=q2, in_=qkv[r0 : r0 + 128, :])
                nc.sync.dma_start(out=k2, in_=qkv[HC + r0 : HC + r0 + 128, :])
                nc.sync.dma_start(
                    out=v2, in_=qkv[2 * HC + r0 : 2 * HC + r0 + 128, :]
                )
                # bf16 q/k: matmul streams 1 col/cycle vs 2 for f32;
                # casts on ACT keep the DVE exp stream unobstructed
                nc.scalar.copy(q2b, q2)
                nc.scalar.copy(k2b, k2)

            for hh in range(2):
                h = pair * 2 + hh
                o = hh * 64
                q = q2b[o : o + 64, :]
                k = k2b[o : o + 64, :]
                v = v2[o : o + 64, :]

                # v -> [s, c] transposes happen on the PE, but are emitted
                # inside chunks 0-1 below so they sit in PE slack during the
                # first exps instead of blocking QK_0 at the head boundary
                trans = ps_av.tile([128, NCH, 64], f32, tag="av")
                vt = vt_pool.tile([128, NCH, 66], bf16)

                def emit_trans(lo, hi):
                    for j in range(lo, hi):
                        nc.tensor.transpose(
                            trans[:, j, :],
                            v[:, j * 128 : (j + 1) * 128],
                            ident[o : o + 64, :],
                        )
                    if hi == NCH:
                        # vt rows: 0..63 = v, 64 = ones (l via the AV matmul)
                        nc.vector.tensor_copy(vt[:, :, 0:64], trans)
                        nc.vector.memset(vt[:, :, 64:65], 1.0)

                av = ps_av.tile([128, T], f32, tag="av")

                def emit_av_half(j, pts_j, half):
                    t0 = half * THALF
                    for qq in range(2):
                        nc.tensor.matmul(
                            av[0:65, t0 + qq * 512 : t0 + (qq + 1) * 512],
                            vt[:, j, 0:65],
                            pts_j[half][:, qq * 512 : (qq + 1) * 512],
                            start=(j == 0),
                            stop=(j == NCH - 1),
                            skip_group_check=True,
                        )

                # software pipeline: QK(j)+exp(j) stream; AV's first
                # half lags one chunk, its second half two chunks, so
                # neither ever waits on the single-engine exp pair
                prev_pts = None
                prev2_pts = None
                for j in range(NCH):
                    kj = k[:, j * 128 : (j + 1) * 128]
                    scs = []
                    for half in range(2):
                        t0 = half * THALF
                        sc = ps_sc.tile([128, THALF], f32, tag="sc")
                        scs.append(sc)
                        for qq in range(2):
                            nc.tensor.matmul(
                                sc[:, qq * 512 : (qq + 1) * 512],
                                kj,
                                q[:, t0 + qq * 512 : t0 + (qq + 1) * 512],
                                start=True,
                                stop=True,
                            )
                    pts = []
                    for half in range(2):
                        pt = pt_pool.tile([128, THALF], bf16)
                        pts.append(pt)
                        if j % 2 == 0:
                            nc.scalar.activation(pt, scs[half], Exp, scale=0.125)
                        else:
                            # Schraudolph exp on DVE halves ACT's load so
                            # the AV matmuls stop waiting on softmax
                            nc.vector.tensor_scalar(
                                pt.bitcast(i16), scs[half], SCH_A, SCH_B,
                                mul_op, add_op,
                            )
                    if j == 0:
                        emit_trans(0, NCH // 2)
                    elif j == 1:
                        emit_trans(NCH // 2, NCH)
                    if prev_pts is not None:
                        emit_av_half(j - 1, prev_pts, 0)
                    if prev2_pts is not None:
                        emit_av_half(j - 2, prev2_pts, 1)
                    prev2_pts = prev_pts
                    prev_pts = pts
                emit_av_half(NCH - 1, prev_pts, 0)
                emit_av_half(NCH - 2, prev2_pts, 1)
                emit_av_half(NCH - 1, prev_pts, 1)

                # evacuate av to SBUF on ACT (it has slack once exp is
                # split; keeps multi-us work out of the DVE queue so the
                # next head's DVE exps start on time)
                av_sb = out_pool.tile([65, T], f32, tag="avsb")
                nc.scalar.copy(av_sb[:, 0:THALF], av[0:65, 0:THALF])
                nc.scalar.copy(av_sb[:, THALF:T], av[0:65, THALF:T])
                # normalize out = av[0:64] * (1/l): the DVE only stages l
                # to partition 0 and takes the [1,*] reciprocal (short);
                # broadcast + multiply run on the otherwise idle gpsimd
                l_sb = l_pool.tile([1, T], f32, tag="lsb")
                rl1 = l_pool.tile([1, T], f32, tag="rl1")
                rlb = l_pool.tile([64, T], f32, tag="rlb")
                o_sb = out_pool.tile([64, T], f32, tag="osb")
                for half in range(2):
                    t0, t1 = half * THALF, (half + 1) * THALF
                    nc.vector.tensor_copy(l_sb[:, t0:t1], av_sb[64:65, t0:t1])
                    nc.vector.reciprocal_approx_fast(
                        out=rl1[:, t0:t1], in_=l_sb[:, t0:t1]
                    )
                    nc.gpsimd.partition_broadcast(rlb[:, t0:t1], rl1[:, t0:t1])
                    nc.gpsimd.tensor_mul(
                        o_sb[:, t0:t1], av_sb[0:64, t0:t1], rlb[:, t0:t1]
                    )
                    nc.sync.dma_start(
                        out=out[h * 64 : (h + 1) * 64, t0:t1], in_=o_sb[:, t0:t1]
                    )

    nc.finalize()
    return nc


def _get_nc():
    if "nc" not in _CACHE:
        _CACHE["nc"] = _build_nc()
    return _CACHE["nc"]


def _run(qkv_full, trace=False, tmpdir=None):
    """qkv_full: (8, 1536, 2048) f32. Returns (out (8,512,2048) f32, exec_ns)."""
    from concourse.bass_utils import run_bass_kernel_spmd

    nc = _get_nc()
    qkv_full = np.ascontiguousarray(np.asarray(qkv_full, dtype=np.float32))
    in_maps = [{"qkv": qkv_full[i]} for i in range(B)]
    res = run_bass_kernel_spmd(
        nc, in_maps, core_ids=list(range(B)), trace=trace, tmpdir=tmpdir
    )
    outs = np.stack([np.asarray(res.results[i]["out"]) for i in range(B)], axis=0)
    return outs, res.exec_time_ns


def kernel(qkv, n_heads=8):
    out, _ = _run(qkv)
    return out.astype(np.float32)

